# revision 10
# baseline (speedup 1.0000x reference)
"""MoE (top-2 of 8 experts, SwiGLU FFN) Trainium2 kernel.

Strategy: hybrid expert-group x tensor-parallel sharding on 8 cores.
----------------------------------------------------------------------
Host router (softmax + top-2 + renorm, ~0.03% of FLOPs) assigns tokens to
experts. Experts are sorted by token count and paired: slot i holds the
i-th largest of each half. Group g (g=0,1) owns 4 experts (one per slot);
the F (ffn) dimension is split into 4 quarters (1408 = 11x128 columns).
Core (g, q) processes group g's tokens against the q-th F-quarter of its
4 experts' w1/w3/w2. This balances per-core work to ~max-pair-sum/2
(vs max-expert-count for pure expert-parallel) and keeps every weight
byte on exactly one core.

Device (per core, identical SPMD program), all operands bf16, PSUM fp32:
  per slot i (4 experts):
    h1T[f,c] = sum_h w1[h,f]*xT[h,c]; h3T likewise (K=128 subtiles)
    actT = silu(h1T)*h3T                  (ACT + DVE, PSUM->SBUF, bf16)
    outT[h,c] = sum_f w2[f,h]*actT[f,c]   (11 f-subtiles = full quarter)
Moving (token) dim is 1 chunk if C<=512 else 2 equal chunks (PSUM bank
limit). Host epilogue: sum the 4 F-quarter partials per group, then
out[token] += combine_weight * y_expert[token].
"""

import math
import numpy as np

import concourse.mybir as mybir
import concourse.tile as tile
from concourse import bacc
from concourse.bass_utils import run_bass_kernel_spmd

E = 8          # experts
TOPK = 2
H = 2048       # hidden
F = 5632       # ffn intermediate
P = 128
N_KT = H // P   # 16 k-subtiles over hidden
N_HT = H // P   # 16 h-tiles
NQ = 4          # F-quarter shards
Q_FT = F // P // NQ  # 11 f-tiles per quarter
NSLOT = 4       # experts per group
NGRP = 2        # expert groups

f32 = mybir.dt.float32
bf16 = mybir.dt.bfloat16

# exposed for test.py (profile/exec time inspection)
LAST_RESULTS = None
LAST_NC = None
LAST_IN_MAPS = None


def _route(x, gate_w):
    """numpy float32 router matching the jax reference (softmax/top2/renorm)."""
    logits = x @ gate_w
    m = logits.max(axis=-1, keepdims=True)
    ex = np.exp(logits - m)
    probs = ex / ex.sum(axis=-1, keepdims=True)
    order = np.argsort(-probs, axis=-1, kind="stable")
    top_idx = order[:, :TOPK]
    top_p = np.take_along_axis(probs, top_idx, axis=-1)
    top_w = top_p / top_p.sum(axis=-1, keepdims=True)
    return top_idx, top_w.astype(np.float32)


def _chunks(C):
    if C <= 512:
        return [(0, C)]
    h = math.ceil(C / 4) * 2
    return [(0, h), (h, C - h)]


def _build(C_slots, reps=1):
    """Per-core SPMD program for slot token-counts C_slots (len 4)."""
    CS = sum(C_slots)
    offs = [sum(C_slots[:i]) for i in range(len(C_slots))]
    n_x = P * N_KT * CS
    n_w13 = NSLOT * 2 * Q_FT * P * N_KT * P
    n_w2 = NSLOT * N_HT * P * Q_FT * P
    nc = bacc.Bacc("TRN2", target_bir_lowering=False, debug=False,
                   enable_asserts=False, num_devices=8)
    inp = nc.dram_tensor("inp", [n_x + n_w13 + n_w2], bf16,
                         kind="ExternalInput").ap()
    xp = inp[0:n_x].rearrange("(p k c) -> p k c", p=P, k=N_KT)
    w13p = inp[n_x:n_x + n_w13].rearrange(
        "(i s j p k q) -> i s j p k q", i=NSLOT, s=2, j=Q_FT, p=P, k=N_KT)
    w2p = inp[n_x + n_w13:].rearrange(
        "(i h p j q) -> i h p j q", i=NSLOT, h=N_HT, p=P, j=Q_FT)
    outp = nc.dram_tensor("outp", [P, N_HT, CS], bf16,
                          kind="ExternalOutput").ap()

    with tile.TileContext(nc) as tc:
        with tc.tile_pool(name="xt", bufs=2) as xt_pool, \
             tc.tile_pool(name="w13", bufs=2) as w13_pool, \
             tc.tile_pool(name="w2", bufs=3) as w2_pool, \
             tc.tile_pool(name="act", bufs=2) as act_pool, \
             tc.tile_pool(name="tmp", bufs=3) as tmp_pool, \
             tc.tile_pool(name="out", bufs=2) as out_pool, \
             tc.tile_pool(name="psum", bufs=2, space="PSUM") as psum_pool:

            for _ in range(reps):
                for i, C in enumerate(C_slots):
                    off = offs[i]
                    chunk_slices = _chunks(C)
                    xt = xt_pool.tile([P, N_KT, C], bf16, tag="xt", name="xt")
                    nc.sync.dma_start(out=xt[:], in_=xp[:, :, off:off + C])
                    a_t = act_pool.tile([P, Q_FT, C], bf16, tag="act",
                                        name="a_t")
                    for j in range(Q_FT):
                        w1b = w13_pool.tile([P, N_KT, P], bf16, tag="w1b",
                                            name="w1b")
                        nc.sync.dma_start(out=w1b[:], in_=w13p[i, 0, j])
                        w3b = w13_pool.tile([P, N_KT, P], bf16, tag="w3b",
                                            name="w3b")
                        nc.sync.dma_start(out=w3b[:], in_=w13p[i, 1, j])
                        for c0, cw in chunk_slices:
                            ps1 = psum_pool.tile([P, 512], f32, tag="ps1",
                                                 name="ps1")[:, :cw]
                            ps3 = psum_pool.tile([P, 512], f32, tag="ps3",
                                                 name="ps3")[:, :cw]
                            for k in range(N_KT):
                                nc.tensor.matmul(ps1, lhsT=w1b[:, k, :],
                                                 rhs=xt[:, k, c0:c0 + cw],
                                                 start=(k == 0),
                                                 stop=(k == N_KT - 1))
                            for k in range(N_KT):
                                nc.tensor.matmul(ps3, lhsT=w3b[:, k, :],
                                                 rhs=xt[:, k, c0:c0 + cw],
                                                 start=(k == 0),
                                                 stop=(k == N_KT - 1))
                            st = tmp_pool.tile([P, 512], f32, tag="silu",
                                               name="st")[:, :cw]
                            nc.scalar.activation(
                                st, ps1, mybir.ActivationFunctionType.Silu)
                            nc.vector.tensor_mul(a_t[:, j, c0:c0 + cw],
                                                 st, ps3)
                    outt = out_pool.tile([P, N_HT, C], bf16, tag="out",
                                         name="outt")
                    for h in range(N_HT):
                        w2b = w2_pool.tile([P, Q_FT, P], bf16, tag="w2b",
                                           name="w2b")
                        nc.sync.dma_start(out=w2b[:], in_=w2p[i, h])
                        for c0, cw in chunk_slices:
                            pso = psum_pool.tile([P, 512], f32, tag="pso",
                                                 name="pso")[:, :cw]
                            for j in range(Q_FT):
                                nc.tensor.matmul(pso, lhsT=w2b[:, j, :],
                                                 rhs=a_t[:, j, c0:c0 + cw],
                                                 start=(j == 0),
                                                 stop=(j == Q_FT - 1))
                            nc.vector.tensor_copy(outt[:, h, c0:c0 + cw], pso)
                    nc.sync.dma_start(out=outp[:, :, off:off + C], in_=outt[:])

    nc.compile()
    return nc


def kernel(hidden_states, gate_w, w1, w3, w2):
    global LAST_RESULTS, LAST_NC, LAST_IN_MAPS
    import os as _os
    import ml_dtypes
    B, S, _ = hidden_states.shape
    x = np.ascontiguousarray(hidden_states.reshape(-1, H), dtype=np.float32)
    gate_w = np.asarray(gate_w, dtype=np.float32)
    w1 = np.asarray(w1, dtype=np.float32)
    w3 = np.asarray(w3, dtype=np.float32)
    w2 = np.asarray(w2, dtype=np.float32)
    T = x.shape[0]

    top_idx, top_w = _route(x, gate_w)

    idx_e, cw_e = [], []
    for e in range(E):
        sel = top_idx == e                     # [T, K] bool; <=1 True per row
        tok = np.nonzero(sel.any(axis=1))[0]
        wgt = top_w[tok, np.argmax(sel[tok], axis=1)]
        idx_e.append(tok)
        cw_e.append(wgt.astype(np.float32))

    # Pair experts by sorted count: slot i = (sorted[2i] -> group0,
    # sorted[2i+1] -> group1). Minimizes sum_i max(pair) = padded work.
    counts = np.array([len(t) for t in idx_e])
    order = np.argsort(-counts, kind="stable")
    grp_experts = [[int(order[2 * i]) for i in range(NSLOT)],
                   [int(order[2 * i + 1]) for i in range(NSLOT)]]
    C_slots = [max(8, int(math.ceil(
        max(counts[grp_experts[0][i]], counts[grp_experts[1][i]]) / 2) * 2))
        for i in range(NSLOT)]

    bf = ml_dtypes.bfloat16
    x_bf = x.astype(bf)
    CS = sum(C_slots)
    offs = [sum(C_slots[:i]) for i in range(NSLOT)]

    # Per-group packed activations; per-(group,slot) weight shards per quarter
    in_maps = [dict() for _ in range(8)]
    for g in range(NGRP):
        xpk = np.zeros((P, N_KT, CS), dtype=bf)
        for i in range(NSLOT):
            ex = grp_experts[g][i]
            tok = idx_e[ex]
            if len(tok):
                xT = np.ascontiguousarray(x_bf[tok].T)       # [H, C_e]
                xpk[:, :, offs[i]:offs[i] + len(tok)] = (
                    xT.reshape(N_KT, P, len(tok)).transpose(1, 0, 2))
        for q in range(NQ):
            core = g * NQ + q
            f0 = q * Q_FT * P
            w13q = np.zeros((NSLOT, 2, Q_FT, P, N_KT, P), dtype=bf)
            w2q = np.zeros((NSLOT, N_HT, P, Q_FT, P), dtype=bf)
            for i in range(NSLOT):
                ex = grp_experts[g][i]
                w13q[i, 0] = (w1[ex][:, f0:f0 + Q_FT * P].astype(bf)
                              .reshape(N_KT, P, Q_FT, P).transpose(2, 1, 0, 3))
                w13q[i, 1] = (w3[ex][:, f0:f0 + Q_FT * P].astype(bf)
                              .reshape(N_KT, P, Q_FT, P).transpose(2, 1, 0, 3))
                w2q[i] = (w2[ex][f0:f0 + Q_FT * P, :].astype(bf)
                          .reshape(Q_FT, P, N_HT, P).transpose(2, 1, 0, 3))
            in_maps[core]["inp"] = np.concatenate(
                [xpk.ravel(), w13q.ravel(), w2q.ravel()])

    # the NTFF trace path needs antenv.axon_hooks, absent in this container;
    # force it off so a stray BASS_TRACE env can't break execution
    _os.environ["BASS_NEVER_TRACE"] = "1"
    nc = _build(C_slots)
    res = run_bass_kernel_spmd(nc, in_maps, list(range(8)))
    LAST_RESULTS = res
    LAST_NC = nc
    LAST_IN_MAPS = in_maps

    out = np.zeros((T, H), dtype=np.float32)
    for g in range(NGRP):
        yg = np.zeros((P, N_HT, CS), dtype=np.float32)
        for q in range(NQ):
            yg += res.results[g * NQ + q]["outp"].astype(np.float32)
        yg = yg.transpose(2, 1, 0).reshape(CS, H)
        for i in range(NSLOT):
            ex = grp_experts[g][i]
            tok = idx_e[ex]
            if not len(tok):
                continue
            out[tok] += cw_e[ex][:, None] * yg[offs[i]:offs[i] + len(tok)]
    return out.reshape(B, S, H)


# ---------------------------------------------------------------------------
# Timing utilities (test-only).
# ---------------------------------------------------------------------------

def make_exec_fn(nc, in_maps):
    """Build a jitted 8-core executor over device-resident inputs for timing."""
    import jax
    import numpy as _np
    from jax.experimental.shard_map import shard_map
    from jax.sharding import Mesh, PartitionSpec, NamedSharding
    from concourse import bass2jax as _b2j
    from concourse.bass2jax import _bass_exec_p, partition_id_tensor
    import concourse.mybir as _mybir

    _b2j.install_neuronx_cc_hook()
    n_cores = len(in_maps)
    partition_name = nc.partition_id_tensor.name if nc.partition_id_tensor else None
    in_names, out_names, out_avals, zero_outs = [], [], [], []
    for alloc in nc.m.functions[0].allocations:
        if not isinstance(alloc, _mybir.MemoryLocationSet):
            continue
        name = alloc.memorylocations[0].name
        if alloc.kind == "ExternalInput":
            if name != partition_name:
                in_names.append(name)
        elif alloc.kind == "ExternalOutput":
            shape = tuple(alloc.tensor_shape)
            dtype = _mybir.dt.np(alloc.dtype)
            out_names.append(name)
            out_avals.append(jax.core.ShapedArray(shape, dtype))
            zero_outs.append(_np.zeros(shape, dtype))
    n_params = len(in_names)
    all_in_names = list(in_names) + list(out_names)
    if partition_name is not None:
        all_in_names.append(partition_name)

    def _body(*args):
        operands = list(args)
        if partition_name is not None:
            operands.append(partition_id_tensor())
        return tuple(_bass_exec_p.bind(
            *operands, out_avals=tuple(out_avals), in_names=tuple(all_in_names),
            out_names=tuple(out_names), lowering_input_output_aliases=(),
            sim_require_finite=True, sim_require_nnan=True, nc=nc))

    devices = jax.devices()[:n_cores]
    mesh = Mesh(_np.asarray(devices), ("core",))
    sh = NamedSharding(mesh, PartitionSpec("core"))
    fn = jax.jit(
        shard_map(_body, mesh=mesh,
                  in_specs=(PartitionSpec("core"),) * (n_params + len(out_names)),
                  out_specs=(PartitionSpec("core"),) * len(out_names),
                  check_rep=False),
        keep_unused=True)
    concat_in = [jax.device_put(_np.concatenate(
        [_np.asarray(in_maps[c][nm]) for c in range(n_cores)], axis=0), sh)
        for nm in in_names]
    concat_zero = [jax.device_put(
        _np.zeros((n_cores * z.shape[0], *z.shape[1:]), z.dtype), sh)
        for z in zero_outs]
    return fn, (*concat_in, *concat_zero)


def async_slope(nc, in_maps, n_lo=16, n_hi=96, tries=6):
    """Per-execution time from the slope of N pipelined async dispatches.
    min-of-tries on both ends rejects shared-host contention spikes."""
    import time as _time
    import jax
    fn, args = make_exec_fn(nc, in_maps)

    def run_n(n):
        t0 = _time.time()
        outs = [fn(*args) for _ in range(n)]
        jax.block_until_ready(outs)
        return _time.time() - t0

    run_n(1)  # warm
    t_lo = min(run_n(n_lo) for _ in range(tries))
    t_hi = min(run_n(n_hi) for _ in range(tries))
    return (t_hi - t_lo) / (n_hi - n_lo)
